# revision 57
# baseline (speedup 1.0000x reference)
"""Biaffine scorer kernel for Trainium2 (Bass/Tile), data-parallel over batch
across 8 NeuronCores, all-bf16 datapath.

Reference computation (per batch item b):
    h = leaky_relu(state @ head_w + head_b)          # (S, BS)
    t = leaky_relu(state @ tail_w + tail_b)          # (S, BS)
    scores1[x,y,o] = h[x] @ U[o] @ t[y]
    scores2[x,y,o] = Wh.h1[x] + Wt.t1[y] + Ww.wemb[x,y] + cls_b
    out = scores1 + scores2                          # (S, S, O)

Device-side decomposition, bf16 everywhere (PSUM fp32), S padded 255->256,
batch items in PAIRS so every matmul streams N=512 moving columns:

    h1T/t1T [128, (bb,x) 512] = Lrelu(w.T @ stateT + bias) -- bias+leaky
        fused into the ACT-engine PSUM evacuation. Feature rows padded to
        128 (120 real + ones-row + zeros); the ones-row comes from
        bias[120]=1 acting on a zero matmul row.
    tu [128, (o, bb, y)]: per o, [U(o).T | Wt + cls_b fold] @ t1T -> one
        contiguous PSUM->SBUF copy per o, alternating ACT/DVE. The A-term
        (Wh.h1, broadcast over y) and cls_b ride inside the ut blocks'
        ones-row/col, so all of scores2 except the width term comes out of
        the bilinear matmul for free.
    out[x, (c,o2,y)] = h1T[:,xtile].T @ tu[:, 2c:2c+2, bb, :] (5 chunks,
        N=512): c0+c1 share a 2-bank PSUM tile evacuated by one ACT op
        (its latency gates the 3-deep PSUM rotation), c2+c3 by one DVE op,
        c4 alternates; each evacuation's output DMA is issued immediately.

The width-embedding term C[x,y,o] = wproj[pos(x,y), o] is batch-independent
and never touches the device: the HOST adds it during output decode
(wproj excludes cls_b so C is zero below the diagonal; cls_b is folded
into the bilinear blocks instead).

Scheduling: a few warmup matmuls on scratch zeros ramp the PE HAM
clock-gate before inputs land; inputs arrive as 4 blob DMAs on the sync
ring whose FIFO order equals consumption order (>=4KB per-partition lines
-- thinner transfers tank ring throughput); pair-1 proj/tu emission is
interleaved into pair-0's finals so the PE never idles at the pair
boundary; outputs leave in 3 slices per tile so the last transfer is
small and the DMA ring is fed as soon as each chunk evacuates.
"""

import numpy as np
import ml_dtypes

import concourse.bass as bass
import concourse.bacc as bacc
import concourse.tile as tile
from concourse import mybir
from concourse.bass_utils import run_bass_kernel_spmd

# problem shape (hardcoded per harness contract)
B, S, H = 32, 255, 1024
BS, WD, O = 120, 20, 10
SP = 256            # padded S
SP2 = 2 * SP        # paired moving dim
KT = H // 128       # 8
NCORES = 8
BPC = B // NCORES   # 4 batch items per core
NP = BPC // 2       # 2 pairs per core
NW = SP * O         # 2560 output cols per (x, b)
NWARM = 12          # PE warmup matmuls

F32 = mybir.dt.float32
BF16 = mybir.dt.bfloat16
NPBF = ml_dtypes.bfloat16

_CACHE: dict = {}


def _emit(tc, d):
    """Emit the per-core program. d: dict of DRAM APs."""
    from contextlib import ExitStack

    nc = tc.nc
    AF = mybir.ActivationFunctionType

    with ExitStack() as ctx:
        const = ctx.enter_context(tc.tile_pool(name="const", bufs=1))
        ht_pool = ctx.enter_context(tc.tile_pool(name="ht", bufs=2))
        tu_pool = ctx.enter_context(tc.tile_pool(name="tu", bufs=2))
        out_pool = ctx.enter_context(tc.tile_pool(name="outp", bufs=6))
        pp_u = ctx.enter_context(tc.tile_pool(name="pp_u", bufs=2, space="PSUM"))
        pp_s = ctx.enter_context(tc.tile_pool(name="pp_s", bufs=3, space="PSUM"))

        # ---- PE warmup: keep HAM at K=8/8 until real matmuls arrive ----
        scratch = const.tile([128, 512], BF16)
        nc.vector.memset(scratch[:], 0.0)
        ps_w = pp_s.tile([128, 1024], F32, tag="ps")
        for wi in range(NWARM):
            nc.tensor.matmul(
                ps_w[:, 0:512],
                lhsT=scratch[:, 0:128],
                rhs=scratch[:],
                start=True,
                stop=True,
            )

        # ---- inputs: 3 blob DMAs on the sync ring, FIFO == priority.
        # Blobs keep >=4KB per-partition lines (thin transfers tank the ring).
        # in1 = sTa(p0)|tw|hw|sTb_kt4 ; in23 = sTb_kt5-7|tb|hb|ut ;
        # in4 = state(p1). The chain reaches kt5 right as in23 lands. ----
        half = KT * SP2 // 2
        in1 = const.tile([128, half + 2560], BF16)
        nc.sync.dma_start(in1[:], d["in1"])
        in23 = const.tile([128, 1538 + O * 128], BF16)
        nc.sync.dma_start(in23[:], d["in23"])
        in4 = const.tile([128, 2 * half], BF16)
        nc.sync.dma_start(in4[:], d["in4"])
        tb = in23[:, 1536:1537]
        hb = in23[:, 1537:1538]
        sb_ut = in23[:, 1538:1538 + O * 128]

        def tw_ap(kt):
            return in1[:, half + kt * 128:half + (kt + 1) * 128]

        def hw_ap(kt):
            return in1[:, half + 1024 + kt * 128:half + 1024 + (kt + 1) * 128]

        def st_ap(p, kt):
            if p == 1:
                return in4[:, kt * 512:(kt + 1) * 512]
            if kt < 4:
                return in1[:, kt * 512:(kt + 1) * 512]
            if kt == 4:
                return in1[:, half + 2048:half + 2560]
            return in23[:, (kt - 5) * 512:(kt - 4) * 512]

        hts = [None] * NP
        tus = [None] * NP
        t1s = [None] * NP

        def proj_half(p, ps, w_ap, lo, hi):
            for kt in range(lo, hi):
                nc.tensor.matmul(
                    ps,
                    lhsT=w_ap(kt),
                    rhs=st_ap(p, kt),
                    start=(kt == 0),
                    stop=(kt == KT - 1),
                )

        def proj_chain(p, ps, w_ap, dst, bv):
            proj_half(p, ps, w_ap, 0, KT)
            nc.scalar.activation(dst[:], ps, AF.Lrelu, bias=bv, alpha=0.01)

        def tu_block(p, o_lo, o_hi):
            tu = tus[p]
            t1T = t1s[p]
            for o in range(o_lo, o_hi):
                ps_u = pp_u.tile([128, SP2], F32, tag="ps_u")
                nc.tensor.matmul(
                    ps_u[:],
                    lhsT=sb_ut[:, o * 128:(o + 1) * 128],
                    rhs=t1T[:],
                    start=True,
                    stop=True,
                )
                if o % 2 == 0:
                    nc.scalar.activation(tu[:, o, :, :], ps_u[:], AF.Copy)
                else:
                    nc.vector.tensor_copy(tu[:, o, :, :], ps_u[:])

        _ti = [0]

        def final_part1(p, bb, xt):
            """Chunks c0,c1 of a tile: matmuls, split evac, first DMA.
            Returns state for final_part2."""
            h1T = hts[p]
            tu = tus[p]
            sb_out = out_pool.tile([128, NW], BF16, tag="sbo")
            lo = bb * SP + xt * 128
            ps_a = pp_s.tile([128, 1024], F32, tag="ps")
            for c, off in ((0, 0), (1, 512)):
                nc.tensor.matmul(
                    ps_a[:, off:off + 512],
                    lhsT=h1T[:, lo:lo + 128],
                    rhs=tu[:, 2 * c:2 * c + 2, bb, :],
                    start=True,
                    stop=True,
                )
            # e0 gates the PSUM rotation -> single op on ACT (fastest PSUM path)
            nc.scalar.activation(sb_out[:, 0:1024], ps_a[:], AF.Copy)
            nc.sync.dma_start(
                d["out"][2 * p + bb, xt, :, 0:1024], sb_out[:, 0:1024]
            )
            return sb_out, lo

        def final_part2(p, bb, xt, sb_out, lo, last=False):
            ti = _ti[0]
            _ti[0] += 1
            h1T = hts[p]
            tu = tus[p]
            ps_b = pp_s.tile([128, 1024], F32, tag="ps")
            ps_c = pp_u.tile([128, 512], F32, tag="ps_u")
            for c, (dst, off) in (
                (2, (ps_b, 0)), (3, (ps_b, 512)), (4, (ps_c, 0))
            ):
                nc.tensor.matmul(
                    dst[:, off:off + 512],
                    lhsT=h1T[:, lo:lo + 128],
                    rhs=tu[:, 2 * c:2 * c + 2, bb, :],
                    start=True,
                    stop=True,
                )
            # e1 has >=2 tiles of rotation slack -> single op on DVE
            nc.vector.tensor_copy(sb_out[:, 1024:2048], ps_b[:])
            if last:
                # ship the middle slice before the final chunk evacuates so
                # the very last transfer is only 128KB
                nc.sync.dma_start(
                    d["out"][2 * p + bb, xt, :, 1024:2048], sb_out[:, 1024:2048]
                )
            if ti % 2 == 0:
                nc.scalar.activation(sb_out[:, 2048:2560], ps_c[:], AF.Copy)
            else:
                nc.vector.tensor_copy(sb_out[:, 2048:2560], ps_c[:])
            if last:
                nc.sync.dma_start(
                    d["out"][2 * p + bb, xt, :, 2048:2560], sb_out[:, 2048:2560]
                )
            else:
                nc.sync.dma_start(
                    d["out"][2 * p + bb, xt, :, 1024:2560], sb_out[:, 1024:2560]
                )

        def final_tile(p, bb, xt):
            st = final_part1(p, bb, xt)
            final_part2(p, bb, xt, *st)

        # ---- software-pipelined emission: engine FIFOs are in program
        # order, so interleave pair-1 proj/tu into pair-0's finals to keep
        # PE dense while spreading ACT/DVE evacuation load ----
        for p in range(NP):
            hts[p] = ht_pool.tile([128, SP2], BF16, tag="h1T", name=f"h1T_{p}")
            t1s[p] = ht_pool.tile([128, SP2], BF16, tag="t1T", name=f"t1T_{p}")
            tus[p] = tu_pool.tile([128, O, 2, SP], BF16, tag="tu", name=f"tu_{p}")

        # pair 0: interleave the two chains' in1-only first halves so the
        # PE has work while blob in2 (sTb|hw) is still in flight
        ps_p0 = pp_s.tile([128, 1024], F32, tag="ps")
        proj_half(0, ps_p0[:, 0:512], tw_ap, 0, 4)
        proj_half(0, ps_p0[:, 512:1024], hw_ap, 0, 4)
        proj_half(0, ps_p0[:, 0:512], tw_ap, 4, KT)
        nc.scalar.activation(t1s[0][:], ps_p0[:, 0:512], AF.Lrelu, bias=tb, alpha=0.01)
        proj_half(0, ps_p0[:, 512:1024], hw_ap, 4, KT)
        nc.scalar.activation(hts[0][:], ps_p0[:, 512:1024], AF.Lrelu, bias=hb, alpha=0.01)
        tu_block(0, 0, 6)
        s000 = final_part1(0, 0, 0)
        tu_block(0, 6, O)
        final_part2(0, 0, 0, *s000)
        s001 = final_part1(0, 0, 1)
        ps_t1 = pp_u.tile([128, SP2], F32, tag="ps_u")
        proj_chain(1, ps_t1[:], tw_ap, t1s[1], tb)
        final_part2(0, 0, 1, *s001)
        s010 = final_part1(0, 1, 0)
        ps_h1 = pp_u.tile([128, SP2], F32, tag="ps_u")
        proj_chain(1, ps_h1[:], hw_ap, hts[1], hb)
        final_part2(0, 1, 0, *s010)
        # stagger pair-1's finals through its tu blocks so output
        # production stays under the ~358GB/s DMA ceiling (no end backlog)
        tu_block(1, 0, 4)
        final_tile(0, 1, 1)
        s100 = final_part1(1, 0, 0)
        tu_block(1, 4, 7)
        s101 = final_part1(1, 0, 1)
        tu_block(1, 7, O)
        final_part2(1, 0, 0, *s100)
        s110 = final_part1(1, 1, 0)
        final_part2(1, 0, 1, *s101)
        s111 = final_part1(1, 1, 1)
        final_part2(1, 1, 0, *s110)
        final_part2(1, 1, 1, *s111, last=True)


def build_nc():
    if "nc" in _CACHE:
        return _CACHE["nc"]
    nc = bacc.Bacc(
        "TRN2", target_bir_lowering=False, debug=False, num_devices=NCORES
    )
    d = {}
    half = KT * SP2 // 2
    d["in1"] = nc.dram_tensor(
        "in1", [128, half + 2560], BF16, kind="ExternalInput"
    ).ap()
    d["in23"] = nc.dram_tensor(
        "in23", [128, 1538 + O * 128], BF16, kind="ExternalInput"
    ).ap()
    d["in4"] = nc.dram_tensor(
        "in4", [128, 2 * half], BF16, kind="ExternalInput"
    ).ap()
    d["out"] = nc.dram_tensor(
        "out", [BPC, 2, 128, NW], BF16, kind="ExternalOutput"
    ).ap()

    with tile.TileContext(nc) as tc:
        _emit(tc, d)
    nc.compile()
    _CACHE["nc"] = nc
    return nc


def prep_inputs(inputs):
    """Host-side packing + transposes + bf16 conversion. Returns dict of np
    arrays shared across cores (stateT is full-batch; shard before dispatch),
    plus the host-side C addend under key "_C"."""
    state = np.asarray(inputs["state"], np.float32)
    head_w = np.asarray(inputs["head_w"], np.float32)
    head_b = np.asarray(inputs["head_b"], np.float32)
    tail_w = np.asarray(inputs["tail_w"], np.float32)
    tail_b = np.asarray(inputs["tail_b"], np.float32)
    U = np.asarray(inputs["U"], np.float32)
    width_table = np.asarray(inputs["width_table"], np.float32)
    cls_w = np.asarray(inputs["cls_w"], np.float32)
    cls_b = np.asarray(inputs["cls_b"], np.float32)
    BSE = BS + 1

    # stateT paired pack: [B/2, 128, (kt, b01, y)], y zero-padded to 256
    stateT = np.zeros((B, H, SP), NPBF)
    stateT[:, :, :S] = state.transpose(0, 2, 1).astype(NPBF)
    # [B/2, 2, KT, 128, SP] -> [B/2, 128, KT, 2, SP]
    stateT = stateT.reshape(B // 2, 2, KT, 128, SP).transpose(0, 3, 2, 1, 4)
    stateT = np.ascontiguousarray(stateT.reshape(B // 2, 128, KT * SP2))

    # head/tail weights: [128, (kt, j)] with j padded 120->128 (zeros)
    hw_sb = np.zeros((128, KT, 128), np.float32)
    hw_sb[:, :, :BS] = head_w.reshape(KT, 128, BS).transpose(1, 0, 2)
    tw_sb = np.zeros((128, KT, 128), np.float32)
    tw_sb[:, :, :BS] = tail_w.reshape(KT, 128, BS).transpose(1, 0, 2)
    hw_sb = hw_sb.reshape(128, KT * 128).astype(NPBF)
    tw_sb = tw_sb.reshape(128, KT * 128).astype(NPBF)

    # ut blocks [j, (o, i)], j/i padded to 128.
    # block[j, o, i] = U[o, i, j];  col i=120 = Wt_ext[o, j] (B-term);
    # row j=120 += Wh_ext[o, i] (A-term; t1 row 120 == 1);
    # [120, o, 120] += cls_b[o].
    ut = np.zeros((128, O, 128), np.float32)
    ut[:BS, :, :BS] = U.transpose(2, 0, 1)
    ut[:BSE, :, BS] = cls_w[:, BS + 1:2 * BSE].T
    ut[BS, :, :BSE] += cls_w[:, :BSE]
    ut[BS, :, BS] += cls_b
    ut = ut.reshape(128, O * 128).astype(NPBF)

    # biases [128, 2] bf16: col0 tail, col1 head; row 120 = 1.0 (ones feature)
    bias = np.zeros((128, 2), np.float32)
    bias[:BS, 0] = tail_b
    bias[:BS, 1] = head_b
    bias[BS, :] = 1.0
    bias = bias.astype(NPBF)

    # host-side C addend [S, S, O] (width term; wproj[0] = 0 by padding_idx)
    pos = np.arange(S)[None, :] - np.arange(S)[:, None] + 1
    pos = pos * (pos > 0)                                 # [S, S]
    wproj = width_table @ cls_w[:, 2 * BSE:].T            # [256, O]
    cadd = wproj[pos]                                     # [S, S, O] fp32

    return {
        "stateT": stateT,
        "hw": hw_sb,
        "tw": tw_sb,
        "ut": ut,
        "bias": bias,
        "_C": cadd,
    }


def run(inputs, trace=False, trace_kwargs=None):
    nc = build_nc()
    full = prep_inputs(inputs)
    cadd = full.pop("_C")
    stateT = full["stateT"]
    in_maps = []
    tw, hw, bias = full["tw"], full["hw"], full["bias"]
    for c in range(NCORES):
        p0 = stateT[c * NP]
        p1 = stateT[c * NP + 1]
        m = {
            "in1": np.ascontiguousarray(
                np.concatenate([p0[:, 0:2048], tw, hw, p0[:, 2048:2560]], axis=1)
            ),
            "in23": np.ascontiguousarray(
                np.concatenate([p0[:, 2560:4096], bias, full["ut"]], axis=1)
            ),
            "in4": p1,
        }
        in_maps.append(m)
    res = run_bass_kernel_spmd(
        nc,
        in_maps,
        core_ids=list(range(NCORES)),
        trace=trace,
        **(trace_kwargs or {}),
    )
    out = np.concatenate([r["out"] for r in res.results], axis=0)
    # [B, xt, p, c, o2, y] -> [B, x, y, o]
    out = out.reshape(B, 2, 128, 5, 2, SP).transpose(0, 1, 2, 5, 3, 4)
    out = out.reshape(B, SP, SP, O)[:, :S, :S, :].astype(np.float32)
    out += cadd[None]
    return out, res


def kernel(**inputs):
    out, _ = run(inputs, trace=False)
    return out


if __name__ == "__main__":
    build_nc()
    print("build ok")


# revision 58
# speedup vs baseline: 1.0056x; 1.0056x over previous
"""Biaffine scorer kernel for Trainium2 (Bass/Tile), data-parallel over batch
across 8 NeuronCores, all-bf16 datapath.

Reference computation (per batch item b):
    h = leaky_relu(state @ head_w + head_b)          # (S, BS)
    t = leaky_relu(state @ tail_w + tail_b)          # (S, BS)
    scores1[x,y,o] = h[x] @ U[o] @ t[y]
    scores2[x,y,o] = Wh.h1[x] + Wt.t1[y] + Ww.wemb[x,y] + cls_b
    out = scores1 + scores2                          # (S, S, O)

Device-side decomposition, bf16 everywhere (PSUM fp32), S padded 255->256,
batch items in PAIRS so every matmul streams N=512 moving columns:

    h1T/t1T [128, (bb,x) 512] = Lrelu(w.T @ stateT + bias) -- bias+leaky
        fused into the ACT-engine PSUM evacuation. Feature rows padded to
        128 (120 real + ones-row + zeros); the ones-row comes from
        bias[120]=1 acting on a zero matmul row.
    tu [128, (o, bb, y)]: per o, [U(o).T | Wt + cls_b fold] @ t1T -> one
        contiguous PSUM->SBUF copy per o, alternating ACT/DVE. The A-term
        (Wh.h1, broadcast over y) and cls_b ride inside the ut blocks'
        ones-row/col, so all of scores2 except the width term comes out of
        the bilinear matmul for free.
    out[x, (c,o2,y)] = h1T[:,xtile].T @ tu[:, 2c:2c+2, bb, :] (5 chunks,
        N=512): c0+c1 share a 2-bank PSUM tile evacuated by one ACT op
        (its latency gates the 3-deep PSUM rotation), c2+c3 by one DVE op,
        c4 alternates; each evacuation's output DMA is issued immediately.

The width-embedding term C[x,y,o] = wproj[pos(x,y), o] is batch-independent
and never touches the device: the HOST adds it during output decode
(wproj excludes cls_b so C is zero below the diagonal; cls_b is folded
into the bilinear blocks instead).

Scheduling: a few warmup matmuls on scratch zeros ramp the PE HAM
clock-gate before inputs land; inputs arrive as 4 blob DMAs on the sync
ring whose FIFO order equals consumption order (>=4KB per-partition lines
-- thinner transfers tank ring throughput); pair-1 proj/tu emission is
interleaved into pair-0's finals so the PE never idles at the pair
boundary; outputs leave in 3 slices per tile so the last transfer is
small and the DMA ring is fed as soon as each chunk evacuates.
"""

import numpy as np
import ml_dtypes

import concourse.bass as bass
import concourse.bacc as bacc
import concourse.tile as tile
from concourse import mybir
from concourse.bass_utils import run_bass_kernel_spmd

# problem shape (hardcoded per harness contract)
B, S, H = 32, 255, 1024
BS, WD, O = 120, 20, 10
SP = 256            # padded S
SP2 = 2 * SP        # paired moving dim
KT = H // 128       # 8
NCORES = 8
BPC = B // NCORES   # 4 batch items per core
NP = BPC // 2       # 2 pairs per core
NW = SP * O         # 2560 output cols per (x, b)
NWARM = 11          # PE warmup matmuls

F32 = mybir.dt.float32
BF16 = mybir.dt.bfloat16
NPBF = ml_dtypes.bfloat16

_CACHE: dict = {}


def _emit(tc, d):
    """Emit the per-core program. d: dict of DRAM APs."""
    from contextlib import ExitStack

    nc = tc.nc
    AF = mybir.ActivationFunctionType

    with ExitStack() as ctx:
        const = ctx.enter_context(tc.tile_pool(name="const", bufs=1))
        ht_pool = ctx.enter_context(tc.tile_pool(name="ht", bufs=2))
        tu_pool = ctx.enter_context(tc.tile_pool(name="tu", bufs=2))
        out_pool = ctx.enter_context(tc.tile_pool(name="outp", bufs=6))
        pp_u = ctx.enter_context(tc.tile_pool(name="pp_u", bufs=2, space="PSUM"))
        pp_s = ctx.enter_context(tc.tile_pool(name="pp_s", bufs=3, space="PSUM"))

        # ---- PE warmup: keep HAM at K=8/8 until real matmuls arrive ----
        scratch = const.tile([128, 512], BF16)
        nc.vector.memset(scratch[:], 0.0)
        ps_w = pp_s.tile([128, 1024], F32, tag="ps")
        for wi in range(NWARM):
            nc.tensor.matmul(
                ps_w[:, 0:512],
                lhsT=scratch[:, 0:128],
                rhs=scratch[:],
                start=True,
                stop=True,
            )

        # ---- inputs: 4 blob DMAs on the sync ring, FIFO == priority.
        # Blobs keep >=4KB per-partition lines (thin transfers tank the ring).
        # in1 = sTa(p0)|tw|hw ; in2 = sTb(p0)|tb|hb ; ut ; in4 = state(p1) ----
        half = KT * SP2 // 2
        in1 = const.tile([128, half + 2048], BF16)
        nc.sync.dma_start(in1[:], d["in1"])
        in2 = const.tile([128, half + 2], BF16)
        nc.sync.dma_start(in2[:], d["in2"])
        sb_ut = const.tile([128, O * 128], BF16)
        nc.sync.dma_start(sb_ut[:], d["ut"])
        in4 = const.tile([128, 2 * half], BF16)
        nc.sync.dma_start(in4[:], d["in4"])
        tb = in2[:, half:half + 1]
        hb = in2[:, half + 1:half + 2]

        def tw_ap(kt):
            return in1[:, half + kt * 128:half + (kt + 1) * 128]

        def hw_ap(kt):
            return in1[:, half + 1024 + kt * 128:half + 1024 + (kt + 1) * 128]

        def st_ap(p, kt):
            if p == 1:
                return in4[:, kt * 512:(kt + 1) * 512]
            if kt < 4:
                return in1[:, kt * 512:(kt + 1) * 512]
            return in2[:, (kt - 4) * 512:(kt - 3) * 512]

        hts = [None] * NP
        tus = [None] * NP
        t1s = [None] * NP

        def proj_half(p, ps, w_ap, lo, hi):
            for kt in range(lo, hi):
                nc.tensor.matmul(
                    ps,
                    lhsT=w_ap(kt),
                    rhs=st_ap(p, kt),
                    start=(kt == 0),
                    stop=(kt == KT - 1),
                )

        def proj_chain(p, ps, w_ap, dst, bv):
            proj_half(p, ps, w_ap, 0, KT)
            nc.scalar.activation(dst[:], ps, AF.Lrelu, bias=bv, alpha=0.01)

        def tu_block(p, o_lo, o_hi):
            tu = tus[p]
            t1T = t1s[p]
            for o in range(o_lo, o_hi):
                ps_u = pp_u.tile([128, SP2], F32, tag="ps_u")
                nc.tensor.matmul(
                    ps_u[:],
                    lhsT=sb_ut[:, o * 128:(o + 1) * 128],
                    rhs=t1T[:],
                    start=True,
                    stop=True,
                )
                if o % 2 == 0:
                    nc.scalar.activation(tu[:, o, :, :], ps_u[:], AF.Copy)
                else:
                    nc.vector.tensor_copy(tu[:, o, :, :], ps_u[:])

        _ti = [0]

        def final_part1(p, bb, xt):
            """Chunks c0,c1 of a tile: matmuls, split evac, first DMA.
            Returns state for final_part2."""
            h1T = hts[p]
            tu = tus[p]
            sb_out = out_pool.tile([128, NW], BF16, tag="sbo")
            lo = bb * SP + xt * 128
            ps_a = pp_s.tile([128, 1024], F32, tag="ps")
            for c, off in ((0, 0), (1, 512)):
                nc.tensor.matmul(
                    ps_a[:, off:off + 512],
                    lhsT=h1T[:, lo:lo + 128],
                    rhs=tu[:, 2 * c:2 * c + 2, bb, :],
                    start=True,
                    stop=True,
                )
            # e0 gates the PSUM rotation -> single op on ACT (fastest PSUM path)
            nc.scalar.activation(sb_out[:, 0:1024], ps_a[:], AF.Copy)
            nc.sync.dma_start(
                d["out"][2 * p + bb, xt, :, 0:1024], sb_out[:, 0:1024]
            )
            return sb_out, lo

        def final_part2(p, bb, xt, sb_out, lo, last=False):
            ti = _ti[0]
            _ti[0] += 1
            h1T = hts[p]
            tu = tus[p]
            ps_b = pp_s.tile([128, 1024], F32, tag="ps")
            ps_c = pp_u.tile([128, 512], F32, tag="ps_u")
            for c, (dst, off) in (
                (2, (ps_b, 0)), (3, (ps_b, 512)), (4, (ps_c, 0))
            ):
                nc.tensor.matmul(
                    dst[:, off:off + 512],
                    lhsT=h1T[:, lo:lo + 128],
                    rhs=tu[:, 2 * c:2 * c + 2, bb, :],
                    start=True,
                    stop=True,
                )
            # e1 has >=2 tiles of rotation slack -> single op on DVE
            nc.vector.tensor_copy(sb_out[:, 1024:2048], ps_b[:])
            if last:
                # ship the middle slice before the final chunk evacuates so
                # the very last transfer is only 128KB
                nc.sync.dma_start(
                    d["out"][2 * p + bb, xt, :, 1024:2048], sb_out[:, 1024:2048]
                )
            if ti % 2 == 0:
                nc.scalar.activation(sb_out[:, 2048:2560], ps_c[:], AF.Copy)
            else:
                nc.vector.tensor_copy(sb_out[:, 2048:2560], ps_c[:])
            if last:
                nc.sync.dma_start(
                    d["out"][2 * p + bb, xt, :, 2048:2560], sb_out[:, 2048:2560]
                )
            else:
                nc.sync.dma_start(
                    d["out"][2 * p + bb, xt, :, 1024:2560], sb_out[:, 1024:2560]
                )

        def final_tile(p, bb, xt):
            st = final_part1(p, bb, xt)
            final_part2(p, bb, xt, *st)

        # ---- software-pipelined emission: engine FIFOs are in program
        # order, so interleave pair-1 proj/tu into pair-0's finals to keep
        # PE dense while spreading ACT/DVE evacuation load ----
        for p in range(NP):
            hts[p] = ht_pool.tile([128, SP2], BF16, tag="h1T", name=f"h1T_{p}")
            t1s[p] = ht_pool.tile([128, SP2], BF16, tag="t1T", name=f"t1T_{p}")
            tus[p] = tu_pool.tile([128, O, 2, SP], BF16, tag="tu", name=f"tu_{p}")

        # pair 0: interleave the two chains' in1-only first halves so the
        # PE has work while blob in2 (sTb|hw) is still in flight
        ps_p0 = pp_s.tile([128, 1024], F32, tag="ps")
        proj_half(0, ps_p0[:, 0:512], tw_ap, 0, 4)
        proj_half(0, ps_p0[:, 512:1024], hw_ap, 0, 4)
        proj_half(0, ps_p0[:, 0:512], tw_ap, 4, KT)
        nc.scalar.activation(t1s[0][:], ps_p0[:, 0:512], AF.Lrelu, bias=tb, alpha=0.01)
        proj_half(0, ps_p0[:, 512:1024], hw_ap, 4, KT)
        nc.scalar.activation(hts[0][:], ps_p0[:, 512:1024], AF.Lrelu, bias=hb, alpha=0.01)
        tu_block(0, 0, 6)
        s000 = final_part1(0, 0, 0)
        tu_block(0, 6, O)
        final_part2(0, 0, 0, *s000)
        s001 = final_part1(0, 0, 1)
        ps_t1 = pp_u.tile([128, SP2], F32, tag="ps_u")
        proj_chain(1, ps_t1[:], tw_ap, t1s[1], tb)
        final_part2(0, 0, 1, *s001)
        s010 = final_part1(0, 1, 0)
        ps_h1 = pp_u.tile([128, SP2], F32, tag="ps_u")
        proj_chain(1, ps_h1[:], hw_ap, hts[1], hb)
        final_part2(0, 1, 0, *s010)
        # stagger pair-1's finals through its tu blocks so output
        # production stays under the ~358GB/s DMA ceiling (no end backlog)
        tu_block(1, 0, 4)
        final_tile(0, 1, 1)
        s100 = final_part1(1, 0, 0)
        tu_block(1, 4, 7)
        s101 = final_part1(1, 0, 1)
        tu_block(1, 7, O)
        final_part2(1, 0, 0, *s100)
        s110 = final_part1(1, 1, 0)
        final_part2(1, 0, 1, *s101)
        s111 = final_part1(1, 1, 1)
        final_part2(1, 1, 0, *s110)
        final_part2(1, 1, 1, *s111, last=True)


def build_nc():
    if "nc" in _CACHE:
        return _CACHE["nc"]
    nc = bacc.Bacc(
        "TRN2", target_bir_lowering=False, debug=False, num_devices=NCORES
    )
    d = {}
    half = KT * SP2 // 2
    d["in1"] = nc.dram_tensor(
        "in1", [128, half + 2048], BF16, kind="ExternalInput"
    ).ap()
    d["in2"] = nc.dram_tensor(
        "in2", [128, half + 2], BF16, kind="ExternalInput"
    ).ap()
    d["ut"] = nc.dram_tensor("ut", [128, O * 128], BF16, kind="ExternalInput").ap()
    d["in4"] = nc.dram_tensor(
        "in4", [128, 2 * half], BF16, kind="ExternalInput"
    ).ap()
    d["out"] = nc.dram_tensor(
        "out", [BPC, 2, 128, NW], BF16, kind="ExternalOutput"
    ).ap()

    with tile.TileContext(nc) as tc:
        _emit(tc, d)
    nc.compile()
    _CACHE["nc"] = nc
    return nc


def prep_inputs(inputs):
    """Host-side packing + transposes + bf16 conversion. Returns dict of np
    arrays shared across cores (stateT is full-batch; shard before dispatch),
    plus the host-side C addend under key "_C"."""
    state = np.asarray(inputs["state"], np.float32)
    head_w = np.asarray(inputs["head_w"], np.float32)
    head_b = np.asarray(inputs["head_b"], np.float32)
    tail_w = np.asarray(inputs["tail_w"], np.float32)
    tail_b = np.asarray(inputs["tail_b"], np.float32)
    U = np.asarray(inputs["U"], np.float32)
    width_table = np.asarray(inputs["width_table"], np.float32)
    cls_w = np.asarray(inputs["cls_w"], np.float32)
    cls_b = np.asarray(inputs["cls_b"], np.float32)
    BSE = BS + 1

    # stateT paired pack: [B/2, 128, (kt, b01, y)], y zero-padded to 256
    stateT = np.zeros((B, H, SP), NPBF)
    stateT[:, :, :S] = state.transpose(0, 2, 1).astype(NPBF)
    # [B/2, 2, KT, 128, SP] -> [B/2, 128, KT, 2, SP]
    stateT = stateT.reshape(B // 2, 2, KT, 128, SP).transpose(0, 3, 2, 1, 4)
    stateT = np.ascontiguousarray(stateT.reshape(B // 2, 128, KT * SP2))

    # head/tail weights: [128, (kt, j)] with j padded 120->128 (zeros)
    hw_sb = np.zeros((128, KT, 128), np.float32)
    hw_sb[:, :, :BS] = head_w.reshape(KT, 128, BS).transpose(1, 0, 2)
    tw_sb = np.zeros((128, KT, 128), np.float32)
    tw_sb[:, :, :BS] = tail_w.reshape(KT, 128, BS).transpose(1, 0, 2)
    hw_sb = hw_sb.reshape(128, KT * 128).astype(NPBF)
    tw_sb = tw_sb.reshape(128, KT * 128).astype(NPBF)

    # ut blocks [j, (o, i)], j/i padded to 128.
    # block[j, o, i] = U[o, i, j];  col i=120 = Wt_ext[o, j] (B-term);
    # row j=120 += Wh_ext[o, i] (A-term; t1 row 120 == 1);
    # [120, o, 120] += cls_b[o].
    ut = np.zeros((128, O, 128), np.float32)
    ut[:BS, :, :BS] = U.transpose(2, 0, 1)
    ut[:BSE, :, BS] = cls_w[:, BS + 1:2 * BSE].T
    ut[BS, :, :BSE] += cls_w[:, :BSE]
    ut[BS, :, BS] += cls_b
    ut = ut.reshape(128, O * 128).astype(NPBF)

    # biases [128, 2] bf16: col0 tail, col1 head; row 120 = 1.0 (ones feature)
    bias = np.zeros((128, 2), np.float32)
    bias[:BS, 0] = tail_b
    bias[:BS, 1] = head_b
    bias[BS, :] = 1.0
    bias = bias.astype(NPBF)

    # host-side C addend [S, S, O] (width term; wproj[0] = 0 by padding_idx)
    pos = np.arange(S)[None, :] - np.arange(S)[:, None] + 1
    pos = pos * (pos > 0)                                 # [S, S]
    wproj = width_table @ cls_w[:, 2 * BSE:].T            # [256, O]
    cadd = wproj[pos]                                     # [S, S, O] fp32

    return {
        "stateT": stateT,
        "hw": hw_sb,
        "tw": tw_sb,
        "ut": ut,
        "bias": bias,
        "_C": cadd,
    }


def run(inputs, trace=False, trace_kwargs=None):
    nc = build_nc()
    full = prep_inputs(inputs)
    cadd = full.pop("_C")
    stateT = full["stateT"]
    in_maps = []
    tw, hw, bias = full["tw"], full["hw"], full["bias"]
    for c in range(NCORES):
        p0 = stateT[c * NP]
        p1 = stateT[c * NP + 1]
        m = {
            "in1": np.ascontiguousarray(
                np.concatenate([p0[:, 0:2048], tw, hw], axis=1)
            ),
            "in2": np.ascontiguousarray(
                np.concatenate([p0[:, 2048:4096], bias], axis=1)
            ),
            "ut": full["ut"],
            "in4": p1,
        }
        in_maps.append(m)
    res = run_bass_kernel_spmd(
        nc,
        in_maps,
        core_ids=list(range(NCORES)),
        trace=trace,
        **(trace_kwargs or {}),
    )
    out = np.concatenate([r["out"] for r in res.results], axis=0)
    # [B, xt, p, c, o2, y] -> [B, x, y, o]
    out = out.reshape(B, 2, 128, 5, 2, SP).transpose(0, 1, 2, 5, 3, 4)
    out = out.reshape(B, SP, SP, O)[:, :S, :S, :].astype(np.float32)
    out += cadd[None]
    return out, res


def kernel(**inputs):
    out, _ = run(inputs, trace=False)
    return out


if __name__ == "__main__":
    build_nc()
    print("build ok")
